# revision 54
# baseline (speedup 1.0000x reference)
"""AtomToTokenEncoder Trainium2 kernel (8 NeuronCores, SPMD, no collectives).

Strategy: token_idx is sorted, so attention (masked to same-token pairs) is
block-diagonal over token groups and the segment-mean is over contiguous
spans.  We re-shard on the host by *token* boundary (96 tokens per core) and
pack whole tokens into 128-slot bins, so attention is tile-local (128x128)
and everything -- attention, FFN, segment mean -- is core-local.

v2 redesign (engine-balanced):
  - The token-equality mask is folded INTO the scores matmul as extra
    contraction rows: sqrt(BIG)*one_hot(token) rows on both the K and Q side
    add BIG*eq[j,i] to the scores; softmax then suppresses cross-token pairs
    by e^-BIG.  The sparse pair bias rides the same mechanism (one-hot row on
    the K side, per-head bias-valued row on the Q side).  No mask DMA, no
    mask add.
  - Softmax denominators: per-head ones-vector matmuls write column sums into
    partitions {0,32,64,96} of a persistent PSUM tile (memset to 1.0 once so
    the unused rows stay finite), one fast approximate reciprocal over the
    whole tile, then a constant 0/1 "head-broadcast" matmul (E4) expands the
    per-(head,query) reciprocals to all 128 feature rows.
  - Segment mean is applied BEFORE the w_tok projection (matmul
    associativity): out = (seg^T @ r2) @ w_tok with 1/count folded into seg.
  - sigmoid(G) = (1+tanh(G/2))/2 with the halves folded into w_g / w_o, so
    the gate uses the Exp-family activation table; only two activation-table
    loads (exp-set, silu-set) instead of ten.
  - Elementwise work is spread over DVE / Activation / GpSimd(Pool).
"""

import os
import sys
import math
import numpy as np

sys.path.insert(0, "/opt/trn_rl_repo")

NCORES = 8
N_ATOM = 6144
D = 128
H = 4
DH = 32
DFF = 512
DM = 512
NT = 768
TPC = NT // NCORES  # 96 tokens per core
NEG = -1.0e30
EPS = 1e-5
BIGV = math.sqrt(30.0)  # sqrt of the mask additive constant

# weight blob layouts
_WB1 = {}
_off = 0
for _name, _w in [("ident", 128), ("wqk", 512), ("wv", 128),
                  ("wg", 128)]:
    _WB1[_name] = (_off, _off + _w)
    _off += _w
WB1_COLS = _off

_WB2 = {}
_off = 0
for _name, _w in [("wo", 128), ("w1", 512), ("w2", 512), ("w3", 512),
                  ("wtok", 512), ("e4", 128)]:
    _WB2[_name] = (_off, _off + _w)
    _off += _w
WB2_COLS = _off

_NC_CACHE = {}

# risky-feature switches (fallbacks exercised if HW disagrees with sim)
USE_POW = True          # rstd = (var+eps)^-0.5 in one DVE tensor_scalar
USE_RECIP_FAST = True   # custom-DVE approximate reciprocal
ATTNORM_GPSIMD = True   # pav*prb (two PSUM operands) on Pool engine


def _build_nc(nb, ka, loop_n=None):
    import contextlib
    import concourse.bass as bass
    import concourse.bacc as bacc
    import concourse.tile as tile
    from concourse import mybir

    F32 = mybir.dt.float32
    BF16 = mybir.dt.bfloat16
    AF = mybir.ActivationFunctionType
    ALU = mybir.AluOpType

    npad = nb * 128

    nc = bacc.Bacc(
        "TRN2", target_bir_lowering=False, debug=False, num_devices=NCORES
    )

    x_d = nc.declare_dram_parameter("x", [128, nb * D], BF16, isOutput=False)
    seg_d = nc.declare_dram_parameter("seg", [128, nb * TPC], BF16, isOutput=False)
    kaug_d = nc.declare_dram_parameter("kaug", [128, npad], BF16, isOutput=False)
    qaug_d = nc.declare_dram_parameter("qaug", [H, 128, npad], BF16, isOutput=False)
    wb1_d = nc.declare_dram_parameter("wb1", [D, WB1_COLS], BF16, isOutput=False)
    wb2_d = nc.declare_dram_parameter("wb2", [D, WB2_COLS], BF16, isOutput=False)
    out_d = nc.declare_dram_parameter("out", [TPC, DM], F32, isOutput=True)

    # free-dim chunkings of the npad atom axis
    half = ((nb + 1) // 2) * 128          # e.g. nb=7 -> 512
    chunks2 = [(0, half), (half, npad)]   # for Q/K/FFN streaming
    gtiles = [(0, min(4, nb))] + ([(4, nb)] if nb > 4 else [])

    with tile.TileContext(nc) as tc:
        with (
            tc.tile_pool(name="pers", bufs=1) as pers,
            tc.tile_pool(name="rot", bufs=3) as rot,
            tc.tile_pool(name="pbig", bufs=3, space="PSUM") as pbig,
            tc.tile_pool(name="ppack", bufs=1, space="PSUM") as ppack,
            tc.tile_pool(name="psmall", bufs=2, space="PSUM") as psmall,
            tc.tile_pool(name="pprb", bufs=1, space="PSUM") as pprb,
            tc.tile_pool(name="ppers", bufs=1, space="PSUM") as ppers,
            (tc.For_i(0, loop_n, 1) if loop_n else contextlib.nullcontext()),
        ):
            # ---------- persistent SBUF ----------
            x_sb = pers.tile([128, npad], BF16, tag="x")
            qn_fm = pers.tile([128, npad], BF16, tag="qnfm")
            u_sb = pers.tile([128, H * npad], BF16, tag="ut")
            v_sb = pers.tile([128, npad], BF16, tag="v")
            tg = pers.tile([128, npad], BF16, tag="tg")
            att_fm = pers.tile([128, npad], BF16, tag="attfm")
            r1_sb = pers.tile([128, npad], BF16, tag="r1")
            h_fm = pers.tile([128, npad], BF16, tag="hfm")
            h12 = pers.tile([128, 4 * npad], BF16, tag="h12")
            r2_sb = pers.tile([128, npad], BF16, tag="r2")
            seg_sb = pers.tile([128, nb * TPC], BF16, tag="seg")
            kaug_sb = pers.tile([128, npad], BF16, tag="kaug")
            qaug_sb = pers.tile([128, H * npad], BF16, tag="qaug")
            wb1_sb = pers.tile([D, WB1_COLS], BF16, tag="wb1")
            wb2_sb = pers.tile([D, WB2_COLS], BF16, tag="wb2")
            sumr_sb = pers.tile([128, TPC], BF16, tag="sumr")
            et_all = pers.tile([128, nb * 512], BF16, tag="etall")
            ones_col = pers.tile([128, 1], BF16, tag="onesc")
            nc.gpsimd.memset(ones_col[:], 1.0)
            eps_sb = pers.tile([128, 1], F32, tag="epsc")
            nc.gpsimd.memset(eps_sb[:], EPS)

            # persistent PSUM tile for softmax sums; memset to 1.0 so the
            # 124 never-written rows reciprocal to 1.0 (finite)
            psumx = ppers.tile([128, 512], F32, tag="psumx")
            nc.vector.memset(psumx[:], 1.0)

            # 4-tile blocks: softmax denominators batched per block
            blocks = [(bs, min(bs + 4, nb)) for bs in range(0, nb, 4)]

            def wsl1(name):
                lo, hi = _WB1[name]
                return wb1_sb[:, lo:hi]

            def wsl2(name):
                lo, hi = _WB2[name]
                return wb2_sb[:, lo:hi]

            # ---------- input DMAs (dependency order; host pre-packs so every
            # transfer is contiguous per partition) ----------
            nc.sync.dma_start(x_sb[:], x_d[:])
            nc.sync.dma_start(wb1_sb[:], wb1_d[:])
            nc.sync.dma_start(kaug_sb[:], kaug_d[:])
            for h in range(H):
                nc.sync.dma_start(
                    qaug_sb[:, h * npad:(h + 1) * npad], qaug_d[h]
                )
            nc.sync.dma_start(wb2_sb[:], wb2_d[:])
            nc.sync.dma_start(seg_sb[:], seg_d[:])

            AX = mybir.AxisListType

            def ln_apply_transpose(src_sb, ptr, tagp):
                """LN over all nb tiles with instruction-minimal stats:
                Square (Act, table-free) + grouped free-axis reduces (DVE),
                then one Sqrt (Act) + one reciprocal (DVE) for all tiles.
                Apply is one tensor_scalar per tile (Pool) feeding the PE
                transpose."""
                sqs = rot.tile([128, npad], F32, tag="lnsq", name=f"{tagp}sq")
                nc.scalar.activation(sqs[:], src_sb[:], AF.Square)
                sums = rot.tile([128, nb], F32, tag=f"{tagp}s", name=f"{tagp}s")
                nc.vector.tensor_reduce(
                    sums[:], src_sb[:].rearrange("p (t f) -> p t f", t=nb),
                    AX.X, ALU.add,
                )
                sumsq = rot.tile([128, nb], F32, tag=f"{tagp}sq2", name=f"{tagp}sq2")
                nc.vector.tensor_reduce(
                    sumsq[:], sqs[:].rearrange("p (t f) -> p t f", t=nb),
                    AX.X, ALU.add,
                )
                mean = rot.tile([128, nb], F32, tag=f"{tagp}mu", name=f"{tagp}mu")
                nc.vector.tensor_scalar(mean[:], sums[:], 1.0 / D, None, ALU.mult)
                m2 = rot.tile([128, nb], F32, tag=f"{tagp}m2", name=f"{tagp}m2")
                nc.gpsimd.tensor_tensor(m2[:], mean[:], mean[:], ALU.mult)
                var = rot.tile([128, nb], F32, tag=f"{tagp}var", name=f"{tagp}var")
                nc.vector.scalar_tensor_tensor(var[:], sumsq[:], 1.0 / D, m2[:],
                                               ALU.mult, ALU.subtract)
                std = rot.tile([128, nb], F32, tag=f"{tagp}sd", name=f"{tagp}sd")
                nc.scalar.activation(std[:], var[:], AF.Sqrt, bias=eps_sb[:])
                rstd = rot.tile([128, nb], F32, tag=f"{tagp}rs", name=f"{tagp}rs")
                nc.vector.reciprocal(rstd[:], std[:])
                for t in range(nb):
                    sl = slice(t * 128, (t + 1) * 128)
                    am = rot.tile([128, 128], BF16, tag=f"{tagp}am",
                                  name=f"{tagp}am{t}")
                    nc.gpsimd.tensor_scalar(
                        am[:], src_sb[:, sl], mean[:, t:t + 1],
                        rstd[:, t:t + 1], ALU.subtract, ALU.mult,
                    )
                    nc.tensor.transpose(ptr[:, sl], am[:], wsl1("ident"))

            # ---------- stage A: LN1 + transpose to feature-major ----------
            ptrA = ppack.tile([128, npad], BF16, tag="ptr", name="ptrA")
            ln_apply_transpose(x_sb, ptrA, "lnA")
            nc.vector.tensor_copy(qn_fm[:], ptrA[:])

            # ---------- stage B: projections ----------
            # scores are the quadratic form qn^T W_h qn with W_h = wk_h wq_h^T
            # folded on host; U_h = W_h^T qn keeps every matmul operand at
            # partition base 0 (nonzero-base operands wedge the PE at runtime)
            for h in range(H):
                for ci, (cs, ce) in enumerate(chunks2):
                    w = ce - cs
                    pu = pbig.tile([128, 512], F32, tag="pb", name="pu")
                    nc.tensor.matmul(
                        pu[:, :w], wsl1("wqk")[:, h * 128:(h + 1) * 128],
                        qn_fm[:, cs:ce], start=True, stop=True,
                    )
                    nc.vector.tensor_copy(
                        u_sb[:, h * npad + cs:h * npad + ce], pu[:, :w]
                    )
            # V and gate G: atom-major per tile
            for gs, ge in gtiles:
                gsl = slice(gs * 128, ge * 128)
                w = (ge - gs) * 128
                pv = pbig.tile([128, 512], F32, tag="pb")
                for j, t in enumerate(range(gs, ge)):
                    nc.tensor.matmul(
                        pv[:, j * 128:(j + 1) * 128],
                        qn_fm[:, t * 128:(t + 1) * 128], wsl1("wv"),
                        start=True, stop=True,
                    )
                nc.vector.tensor_copy(v_sb[:, gsl], pv[:, :w])
                pg_ = pbig.tile([128, 512], F32, tag="pb")
                for j, t in enumerate(range(gs, ge)):
                    nc.tensor.matmul(
                        pg_[:, j * 128:(j + 1) * 128],
                        qn_fm[:, t * 128:(t + 1) * 128], wsl1("wg"),
                        start=True, stop=True,
                    )
                # wg pre-scaled by g1*0.5 on host: tg = tanh(G/2)
                nc.scalar.activation(tg[:, gsl], pg_[:, :w], AF.Tanh)

            # ---------- stage C: attention ----------
            # scores + exp per tile; softmax denominators / reciprocal /
            # head-broadcast batched per 4-tile block to cut the number of
            # cross-engine hops on the critical path
            qaug_v = qaug_sb[:].rearrange("p (h s) -> p h s", h=H)
            et_v = et_all[:].rearrange("p (t h i) -> p t h i", t=nb, h=H)
            for bs, be in blocks:
                bw = (be - bs) * 128
                for t in range(bs, be):
                    sl = slice(t * 128, (t + 1) * 128)
                    ps = pbig.tile([128, 512], F32, tag="pb")
                    for h in range(H):
                        nc.tensor.matmul(
                            ps[:, h * 128:(h + 1) * 128],
                            u_sb[:, h * npad + t * 128: h * npad + (t + 1) * 128],
                            qn_fm[:, sl],
                            start=True, stop=False, skip_group_check=True,
                        )
                    # all 4 heads' aug contributions in one strided-rhs matmul
                    nc.tensor.matmul(
                        ps[:], kaug_sb[:, sl],
                        qaug_v[:, :, t * 128:(t + 1) * 128],
                        start=False, stop=True, skip_group_check=True,
                    )
                    nc.scalar.activation(et_all[:, t * 512:(t + 1) * 512], ps[:],
                                         AF.Exp)
                # per-block column sums: head h sums land at partition h*32,
                # tile-in-block at column block (t-bs)*128
                for h in range(H):
                    nc.tensor.matmul(
                        psumx[h * DH:h * DH + 1, 0:bw], ones_col[:],
                        et_v[:, bs:be, h, :],
                        start=True, stop=True,
                        tile_position=(0, h * DH),
                    )
                rinvf = rot.tile([128, 512], F32, tag="rinvf")
                if USE_RECIP_FAST:
                    nc.vector.reciprocal_approx_fast(rinvf[:, 0:bw],
                                                     psumx[:, 0:bw])
                else:
                    nc.vector.reciprocal(rinvf[:, 0:bw], psumx[:, 0:bw])
                rinvb = rot.tile([128, 512], BF16, tag="rinvb")
                nc.gpsimd.tensor_copy(rinvb[:, 0:bw], rinvf[:, 0:bw])
                prb_all = pprb.tile([128, 512], F32, tag="prb")
                nc.tensor.matmul(prb_all[:, 0:bw], wsl2("e4"), rinvb[:, 0:bw],
                                 start=True, stop=True)
                prb_sb = rot.tile([128, 512], BF16, tag="prbsb")
                nc.vector.tensor_copy(prb_sb[:, 0:bw], prb_all[:, 0:bw])
                # AV for the whole block into one PSUM tile; one attnorm op
                pq2 = psmall.tile([128, 512], F32, tag="pq")
                for j, tt in enumerate(range(bs, be)):
                    for h in range(H):
                        nc.tensor.matmul(
                            pq2[h * DH:(h + 1) * DH, j * 128:(j + 1) * 128],
                            v_sb[:, tt * 128 + h * DH: tt * 128 + (h + 1) * DH],
                            et_all[:, tt * 512 + h * 128: tt * 512 + (h + 1) * 128],
                            start=True, stop=True,
                            tile_position=(0, h * DH),
                        )
                nc.vector.tensor_tensor(
                    att_fm[:, bs * 128:bs * 128 + bw], pq2[:, 0:bw],
                    prb_sb[:, 0:bw], ALU.mult,
                )

            # ---------- stage D: output proj + tanh-gate + residual ----------
            # wo pre-scaled by 0.5 on host: r1 = x + (1+tanh(G/2)) * (att@wo/2)
            for gs, ge in gtiles:
                gsl = slice(gs * 128, ge * 128)
                w = (ge - gs) * 128
                po = pbig.tile([128, 512], F32, tag="pb")
                for j, t in enumerate(range(gs, ge)):
                    nc.tensor.matmul(
                        po[:, j * 128:(j + 1) * 128],
                        att_fm[:, t * 128:(t + 1) * 128], wsl2("wo"),
                        start=True, stop=True,
                    )
                gu = rot.tile([128, 512], BF16, tag="gu")
                nc.vector.scalar_tensor_tensor(
                    gu[:, :w], tg[:, gsl], 1.0, po[:, :w], ALU.add, ALU.mult
                )
                nc.gpsimd.tensor_tensor(r1_sb[:, gsl], gu[:, :w], x_sb[:, gsl],
                                        ALU.add)

            # ---------- stage E: LN2 + transpose ----------
            ptrE = ppack.tile([128, npad], BF16, tag="ptr", name="ptrE")
            ln_apply_transpose(r1_sb, ptrE, "lnE")
            nc.vector.tensor_copy(h_fm[:], ptrE[:])

            # ---------- stage F: SwiGLU FFN ----------
            for ffc in range(4):
                ws1_ = wsl2("w1")[:, ffc * 128:(ffc + 1) * 128]
                ws2_ = wsl2("w2")[:, ffc * 128:(ffc + 1) * 128]
                for ci, (cs, ce) in enumerate(chunks2):
                    w = ce - cs
                    pf1 = pbig.tile([128, 512], F32, tag="pb")
                    nc.tensor.matmul(pf1[:, :w], ws1_, h_fm[:, cs:ce],
                                     start=True, stop=True)
                    s1 = rot.tile([128, 512], BF16, tag="s1")
                    nc.scalar.activation(s1[:, :w], pf1[:, :w], AF.Silu)
                    pf2 = pbig.tile([128, 512], F32, tag="pb")
                    nc.tensor.matmul(pf2[:, :w], ws2_, h_fm[:, cs:ce],
                                     start=True, stop=True)
                    osl = slice(ffc * npad + cs, ffc * npad + ce)
                    nc.vector.tensor_tensor(h12[:, osl], s1[:, :w],
                                            pf2[:, :w], ALU.mult)
            # w3: atom-major output, two tiles per PSUM tile + one residual op
            t = 0
            while t < nb:
                tp2 = min(t + 2, nb)
                w2_ = (tp2 - t) * 128
                pq3 = psmall.tile([128, 512], F32, tag="pq", name=f"pq3_{t}")
                for j, tt in enumerate(range(t, tp2)):
                    for ffc in range(4):
                        nc.tensor.matmul(
                            pq3[:, j * 128:(j + 1) * 128],
                            h12[:, ffc * npad + tt * 128: ffc * npad + (tt + 1) * 128],
                            wsl2("w3")[:, ffc * 128:(ffc + 1) * 128],
                            start=(ffc == 0), stop=(ffc == 3),
                        )
                nc.vector.tensor_tensor(r2_sb[:, t * 128:t * 128 + w2_],
                                        pq3[:, 0:w2_],
                                        r1_sb[:, t * 128:t * 128 + w2_],
                                        ALU.add)
                t = tp2

            # ---------- stage G: segment-mean then token projection ----------
            # out = (seg^T @ r2) @ wtok ; 1/count folded into seg on host
            psum_seg = ppers.tile([128, 512], F32, tag="psumx", name="pseg")
            for t in range(nb):
                sl = slice(t * 128, (t + 1) * 128)
                nc.tensor.matmul(
                    psum_seg[:, 0:TPC], r2_sb[:, sl],
                    seg_sb[:, t * TPC:(t + 1) * TPC],
                    start=(t == 0), stop=(t == nb - 1),
                )
            nc.scalar.copy(sumr_sb[:], psum_seg[:, 0:TPC])
            pout = pbig.tile([128, 512], F32, tag="pb", name="pout")
            nc.tensor.matmul(pout[:TPC, :], sumr_sb[:], wsl2("wtok"),
                             start=True, stop=True)
            outp = rot.tile([TPC, 512], F32, tag="outp")
            nc.scalar.copy(outp[:], pout[:TPC, :])
            nc.sync.dma_start(out_d[:], outp[:])

    nc.finalize()
    return nc


def get_nc(nb, ka, loop_n=None):
    key = (nb, ka, loop_n)
    if key not in _NC_CACHE:
        _NC_CACHE[key] = _build_nc(nb, ka, loop_n)
    return _NC_CACHE[key]


# --------------------------------------------------------------------------
# host-side preprocessing
# --------------------------------------------------------------------------

def _prep(inputs):
    c_atom = np.ascontiguousarray(np.asarray(inputs["c_atom"], dtype=np.float32))
    p_lm = np.asarray(inputs["p_lm"], dtype=np.float32)
    p_idx = np.asarray(inputs["p_lm_idx"]).astype(np.int64)
    tok = np.asarray(inputs["token_idx"]).astype(np.int64)
    n_tokens = int(np.asarray(inputs["n_tokens"]))

    if c_atom.shape != (N_ATOM, D) or tok.shape != (N_ATOM,) or n_tokens != NT:
        return None
    if np.any(np.diff(tok) < 0) or tok.min() < 0 or tok.max() >= NT:
        return None

    g1 = np.asarray(inputs["ln_attn_g"], np.float32)
    b1 = np.asarray(inputs["ln_attn_b"], np.float32)
    g2 = np.asarray(inputs["ln_ff_g"], np.float32)
    b2 = np.asarray(inputs["ln_ff_b"], np.float32)
    b_tok = np.asarray(inputs["b_tok"], np.float32)
    # the fast path folds LN gamma into the weights; beta / b_tok == 0 in
    # this model family -- fall back to the numpy path otherwise
    if np.any(b1 != 0) or np.any(b2 != 0) or np.any(b_tok != 0):
        return None

    w_q = np.asarray(inputs["w_q"], np.float32)
    w_k = np.asarray(inputs["w_k"], np.float32)
    w_v = np.asarray(inputs["w_v"], np.float32)
    w_g = np.asarray(inputs["w_g"], np.float32)
    w_o = np.asarray(inputs["w_o"], np.float32)
    w_pb = np.asarray(inputs["w_pb"], np.float32)
    b_pb = np.asarray(inputs["b_pb"], np.float32)
    w1 = np.asarray(inputs["w1"], np.float32)
    w2 = np.asarray(inputs["w2"], np.float32)
    w3 = np.asarray(inputs["w3"], np.float32)
    w_tok = np.asarray(inputs["w_tok"], np.float32)

    scale = 1.0 / math.sqrt(DH)
    wq_eff = (g1[:, None] * w_q) * scale
    wk_eff = g1[:, None] * w_k
    wv_eff = g1[:, None] * w_v
    wg_eff = g1[:, None] * w_g * 0.5   # tanh-gate halving
    wo_eff = w_o * 0.5                 # sigmoid = (1+tanh)/2
    w1_eff = g2[:, None] * w1
    w2_eff = g2[:, None] * w2

    counts = np.bincount(tok, minlength=NT)

    # ---- pack whole tokens into 128-slot bins (greedy, per core) ----
    # first pass: find bins needed per core -> nb
    nb = 0
    for c in range(NCORES):
        b, fill = 0, 0
        for t in range(c * TPC, (c + 1) * TPC):
            n = int(counts[t])
            if n == 0:
                continue
            if n > 128:
                return None
            if fill + n > 128:
                b += 1
                fill = 0
            fill += n
        nb = max(nb, b + 1)
    if nb > 16:
        return None
    npad = nb * 128

    x_pad = np.zeros((NCORES, nb, 128, D), np.float32)
    slot_of_atom = np.full(N_ATOM, -1, np.int64)
    tile_tokens = [[[] for _ in range(nb)] for _ in range(NCORES)]
    # token -> (core, bin, tok-row-in-bin); slot ranges
    tok_place = {}
    a = 0
    for c in range(NCORES):
        b, fill = 0, 0
        for t in range(c * TPC, (c + 1) * TPC):
            n = int(counts[t])
            if n == 0:
                continue
            if fill + n > 128:
                b += 1
                fill = 0
            x_pad[c, b, fill:fill + n] = c_atom[a:a + n]
            slot_of_atom[a:a + n] = (c * nb + b) * 128 + fill + np.arange(n)
            tok_place[t] = (c, b, len(tile_tokens[c][b]), fill, n)
            tile_tokens[c][b].append((fill, n))
            fill += n
            a += n
    assert a == N_ATOM

    # ---- surviving pairs (same token), dedup with last-write-wins ----
    tok_i = tok[p_idx[:, 0]]
    tok_j = tok[p_idx[:, 1]]
    keep = np.nonzero(tok_i == tok_j)[0]
    pair_map = {}
    if keep.size:
        bias_vals = p_lm[keep] @ w_pb + b_pb  # (K, H)
        gi = slot_of_atom[p_idx[keep, 0]]  # query side
        gj = slot_of_atom[p_idx[keep, 1]]  # key side
        for n in range(keep.size):
            ci, ri = divmod(int(gi[n]), nb * 128)
            bi, si = divmod(ri, 128)
            cj, rj = divmod(int(gj[n]), nb * 128)
            bj, sj = divmod(rj, 128)
            assert ci == cj and bi == bj
            pair_map[(ci, bi, sj, si)] = bias_vals[n]
    pairs_by_tile = [[[] for _ in range(nb)] for _ in range(NCORES)]
    for (c, b, sj, si), bv in pair_map.items():
        pairs_by_tile[c][b].append((sj, si, bv))

    # ---- aug row budget (rows are zero-padded to a full 128-contraction
    # so the scores+aug accumulation never switches PE tiling mode) ----
    ka = 8
    for c in range(NCORES):
        for b in range(nb):
            need = len(tile_tokens[c][b]) + len(pairs_by_tile[c][b])
            ka = max(ka, need)
    if ka > 128:
        return None
    ka = 128

    kaug = np.zeros((NCORES, ka, npad), np.float32)
    qaug = np.zeros((NCORES, H, ka, npad), np.float32)
    for c in range(NCORES):
        for b in range(nb):
            base = b * 128
            r = 0
            for (fill, n) in tile_tokens[c][b]:
                kaug[c, r, base + fill:base + fill + n] = BIGV
                qaug[c, :, r, base + fill:base + fill + n] = BIGV
                r += 1
            for (sj, si, bv) in pairs_by_tile[c][b]:
                kaug[c, r, base + sj] = 1.0
                qaug[c, :, r, base + si] = bv[:, None][:, 0]
                r += 1

    # ---- segment matrix with 1/count folded in ----
    seg = np.zeros((NCORES, nb, 128, TPC), np.float32)
    for t, (c, b, _tr, fill, n) in tok_place.items():
        tloc = t - c * TPC
        seg[c, b, fill:fill + n, tloc] = 1.0 / n

    # ---- weight blobs ----
    w3_sh = np.ascontiguousarray(
        w3.reshape(4, 128, D).transpose(1, 0, 2).reshape(128, 4 * D)
    )
    ident = np.eye(128, dtype=np.float32)
    e4 = np.zeros((128, 128), np.float32)
    for d_ in range(128):
        e4[(d_ // DH) * DH, d_] = 1.0

    # fold wq/wk into one per-head quadratic-form matrix W_h = wk_h @ wq_h^T
    wqk = np.concatenate(
        [wk_eff[:, h * DH:(h + 1) * DH] @ wq_eff[:, h * DH:(h + 1) * DH].T
         for h in range(H)], axis=1,
    )  # (128, H*128)

    import ml_dtypes
    bf16 = ml_dtypes.bfloat16
    wb1 = np.concatenate([ident, wqk, wv_eff, wg_eff], axis=1).astype(bf16)
    wb2 = np.concatenate([wo_eff, w1_eff, w2_eff, w3_sh, w_tok, e4], axis=1).astype(bf16)
    assert wb1.shape == (D, WB1_COLS) and wb2.shape == (D, WB2_COLS)

    in_maps = []
    for c in range(NCORES):
        # pack [tile, slot, f] -> [slot, tile*f] so DMA lines are contiguous
        x_c = np.ascontiguousarray(
            x_pad[c].transpose(1, 0, 2).reshape(128, nb * D)
        )
        seg_c = np.ascontiguousarray(
            seg[c].transpose(1, 0, 2).reshape(128, nb * TPC)
        )
        in_maps.append({
            "x": x_c.astype(bf16),
            "seg": seg_c.astype(bf16),
            "kaug": np.ascontiguousarray(kaug[c]).astype(bf16),
            "qaug": np.ascontiguousarray(qaug[c]).astype(bf16),
            "wb1": wb1,
            "wb2": wb2,
        })
    return in_maps, nb, ka


# --------------------------------------------------------------------------
# numpy fallback (exact reference port) -- safety net only
# --------------------------------------------------------------------------

def _numpy_reference(**inp):
    def ln(x, g, b, eps=1e-5):
        mu = x.mean(-1, keepdims=True)
        var = x.var(-1, keepdims=True)
        return (x - mu) / np.sqrt(var + eps) * g + b

    c_atom = np.asarray(inp["c_atom"], np.float64)
    tok = np.asarray(inp["token_idx"]).astype(np.int64)
    n_tokens = int(np.asarray(inp["n_tokens"]))
    n_atom = c_atom.shape[0]
    d_h = D // H
    q = c_atom
    q_n = ln(q, np.asarray(inp["ln_attn_g"], np.float64), np.asarray(inp["ln_attn_b"], np.float64))
    Q = (q_n @ np.asarray(inp["w_q"], np.float64)).reshape(n_atom, H, d_h)
    K = (q_n @ np.asarray(inp["w_k"], np.float64)).reshape(n_atom, H, d_h)
    V = (q_n @ np.asarray(inp["w_v"], np.float64)).reshape(n_atom, H, d_h)
    G = q_n @ np.asarray(inp["w_g"], np.float64)
    scores = np.einsum("ihd,jhd->hij", Q, K) / math.sqrt(d_h)
    bias = np.asarray(inp["p_lm"], np.float64) @ np.asarray(inp["w_pb"], np.float64) + np.asarray(inp["b_pb"], np.float64)
    p_idx = np.asarray(inp["p_lm_idx"]).astype(np.int64)
    pair_bias = np.zeros((H, n_atom, n_atom))
    pair_bias[:, p_idx[:, 0], p_idx[:, 1]] = bias.T
    scores = scores + pair_bias
    mask = tok[:, None] == tok[None, :]
    scores = np.where(mask[None], scores, NEG)
    scores -= scores.max(-1, keepdims=True)
    e = np.exp(scores)
    attn = e / e.sum(-1, keepdims=True)
    att_out = np.einsum("hij,jhd->ihd", attn, V).reshape(n_atom, D)
    q = q + (1 / (1 + np.exp(-G))) * (att_out @ np.asarray(inp["w_o"], np.float64))
    h = ln(q, np.asarray(inp["ln_ff_g"], np.float64), np.asarray(inp["ln_ff_b"], np.float64))
    a1 = h @ np.asarray(inp["w1"], np.float64)
    q = q + ((a1 / (1 + np.exp(-a1))) * (h @ np.asarray(inp["w2"], np.float64))) @ np.asarray(inp["w3"], np.float64)
    feat = q @ np.asarray(inp["w_tok"], np.float64) + np.asarray(inp["b_tok"], np.float64)
    sums = np.zeros((n_tokens, DM))
    np.add.at(sums, tok, feat)
    cnt = np.bincount(tok, minlength=n_tokens).astype(np.float64)
    return (sums / np.maximum(cnt, 1.0)[:, None]).astype(np.float32)


# --------------------------------------------------------------------------
# entry points
# --------------------------------------------------------------------------

def _run(nc, in_maps, trace=False, tmpdir=None):
    from concourse.bass_utils import run_bass_kernel_spmd
    return run_bass_kernel_spmd(
        nc, in_maps, core_ids=list(range(NCORES)), trace=trace, tmpdir=tmpdir
    )


# --------------------------------------------------------------------------
# wall-clock benchmarking (no NTFF profiling available under this axon
# build): wrap the kernel body in a For_i loop of K iterations and take the
# wall-time slope between two K values; the per-execute dispatch overhead
# cancels out.
# --------------------------------------------------------------------------

class _BenchExec:
    def __init__(self, nc, in_maps):
        import jax
        import numpy as np
        from jax.sharding import Mesh, PartitionSpec
        from jax.experimental.shard_map import shard_map
        from concourse import bass2jax, mybir

        bass2jax.install_neuronx_cc_hook()
        n_cores = len(in_maps)
        partition_name = (
            nc.partition_id_tensor.name if nc.partition_id_tensor else None
        )
        in_names, out_names, out_avals, zero_outs = [], [], [], []
        for alloc in nc.m.functions[0].allocations:
            if not isinstance(alloc, mybir.MemoryLocationSet):
                continue
            name = alloc.memorylocations[0].name
            if alloc.kind == "ExternalInput":
                if name != partition_name:
                    in_names.append(name)
            elif alloc.kind == "ExternalOutput":
                out_names.append(name)
                shape = tuple(alloc.tensor_shape)
                dtype = mybir.dt.np(alloc.dtype)
                out_avals.append(jax.core.ShapedArray(shape, dtype))
                zero_outs.append(np.zeros(shape, dtype))
        n_params = len(in_names)
        n_outs = len(out_avals)
        in_names_all = in_names + out_names
        if partition_name is not None:
            in_names_all.append(partition_name)
        donate = tuple(range(n_params, n_params + n_outs))

        def _body(*args):
            operands = list(args)
            if partition_name is not None:
                operands.append(bass2jax.partition_id_tensor())
            outs = bass2jax._bass_exec_p.bind(
                *operands,
                out_avals=tuple(out_avals),
                in_names=tuple(in_names_all),
                out_names=tuple(out_names),
                lowering_input_output_aliases=(),
                sim_require_finite=True,
                sim_require_nnan=True,
                nc=nc,
            )
            return tuple(outs)

        devices = jax.devices()[:n_cores]
        mesh = Mesh(np.asarray(devices), ("core",))
        in_specs = (PartitionSpec("core"),) * (n_params + n_outs)
        out_specs = (PartitionSpec("core"),) * len(out_names)
        self.fn = jax.jit(
            shard_map(_body, mesh=mesh, in_specs=in_specs, out_specs=out_specs,
                      check_rep=False),
            donate_argnums=donate, keep_unused=True,
        )
        from jax.sharding import NamedSharding
        sh = NamedSharding(mesh, PartitionSpec("core"))
        concat_in = [
            np.concatenate([np.asarray(in_maps[c][nm]) for c in range(n_cores)], axis=0)
            for nm in in_names
        ]
        self.dev_in = [jax.device_put(x, sh) for x in concat_in]
        self.zero_shapes = [
            ((n_cores * z.shape[0],) + z.shape[1:], z.dtype) for z in zero_outs
        ]
        self.sh = sh
        self.jax = jax
        self.np = np

    def call(self):
        zeros = [self.jax.device_put(self.np.zeros(s, d), self.sh)
                 for s, d in self.zero_shapes]
        out = self.fn(*self.dev_in, *zeros)
        self.jax.block_until_ready(out)
        return out

    def time_it(self, reps=10):
        import time
        self.call()
        ts = []
        for _ in range(reps):
            t0 = time.perf_counter()
            self.call()
            ts.append(time.perf_counter() - t0)
        return min(ts), ts


def benchmark(in_maps, nb, ka, k_lo=2, k_hi=130, reps=14):
    """Interleaved lo/hi wall-clock slope: pairs cancel baseline drift."""
    import time
    ex_lo = _BenchExec(get_nc(nb, ka, loop_n=k_lo), in_maps)
    ex_hi = _BenchExec(get_nc(nb, ka, loop_n=k_hi), in_maps)
    ex_lo.call(); ex_hi.call()
    diffs, ts_lo, ts_hi = [], [], []
    for _ in range(reps):
        t0 = time.perf_counter(); ex_lo.call()
        t1 = time.perf_counter(); ex_hi.call()
        t2 = time.perf_counter()
        ts_lo.append(t1 - t0); ts_hi.append(t2 - t1)
        diffs.append((t2 - t1) - (t1 - t0))
    # interference only ever adds time -> min of paired diffs is the
    # least-biased estimator under multi-tenant jitter
    per_iter = min(diffs) / (k_hi - k_lo)
    return per_iter, min(ts_lo), min(ts_hi), ts_lo, ts_hi


def kernel(**inputs):
    prep = _prep(inputs)
    if prep is None:
        return _numpy_reference(**inputs)
    in_maps, nb, ka = prep
    res = _run(get_nc(nb, ka), in_maps)
    return np.concatenate([res.results[c]["out"] for c in range(NCORES)], axis=0)


def kernel_profiled(**inputs):
    """Returns (output, exec_time_ns, results_obj, nb, ka). Used by test.py."""
    prep = _prep(inputs)
    assert prep is not None
    in_maps, nb, ka = prep
    import tempfile
    tmpdir = tempfile.mkdtemp(prefix="atok_trace_")
    try:
        res = _run(get_nc(nb, ka), in_maps, trace=True, tmpdir=tmpdir)
    except ModuleNotFoundError:
        res = _run(get_nc(nb, ka), in_maps)
    out = np.concatenate([res.results[c]["out"] for c in range(NCORES)], axis=0)
    return out, res.exec_time_ns, res, nb, ka
